# revision 43
# baseline (speedup 1.0000x reference)
"""Multi-head causal attention (B=4, L=2048, D=1024, H=16) on 8 TRN2 NeuronCores.

Sharding: core c handles batch b = c//2 and head-group hg = c%2 (8 heads, 512
dims). Each core computes Q/K/V projections for its heads, causal attention,
and a partial output projection (its 512 input dims of Wo). Host sums the two
partials per batch.

Inputs arrive pre-transposed and pre-cast to fp16 from the host (x^T, Wq^T,
Wk^T/8, Wv^T, Wo^T) — no on-device input transposes or casts, and half the
input DMA traffic; the output is returned fp16 and the partials are summed in
fp32 on the host. Scores near the causal diagonal are trimmed to 128-column
granularity, masking is one triangular [128,128] multiply per diagonal block,
and the softmax epilogue is one reciprocal + one broadcast multiply per
(head, chunk) out of a single 4-q-tile PSUM accumulator. The attention stream
is software-pipelined one head deep — head h's AV/epilogue runs between head
h+1's score matmuls — and within each head the diagonal (small-exp) slots are
emitted first so the 1024-column exp tiles trail across head boundaries,
keeping the scalar engine (the exp bottleneck of the back half) fed.
Projections for later chunks and the output projection drain through a gated,
watermarked filler: explicit watermarks (not just emission order) guarantee
every cross-stage read is emitted after its producer — the Tile framework
only orders reads against PREVIOUSLY-emitted writes. The latency-critical
first DMAs issue round-robin across three engine queues. The final unit
reverses its slot order (full-width first) so its AV unblocks on the small
trailing diagonal exps, the filler pacing keeps the PE fed through the last
slots (p-state stays at full clock into the tail), and the post-exp output
tiles drain through the then-idle scalar engine instead of queueing on DVE.
"""
import sys

sys.path.insert(0, "/opt/trn_rl_repo")

import numpy as np

import concourse.bass as bass
import concourse.mybir as mybir
import concourse.tile as tile
from concourse import bacc
from concourse.masks import make_identity

F32 = mybir.dt.float32
F16 = mybir.dt.float16
MM = F16
AF = mybir.ActivationFunctionType

B, L, D, H = 4, 2048, 1024, 16
DK = 64
E = 512
NL = L // 128
ND = D // 128
NE = E // 128
NJ = L // 512
NK = L // 128
NDO = E // 128

_CACHE = {}


def build_program():
    nc = bacc.Bacc("TRN2", target_bir_lowering=False, debug=False, num_devices=8)

    xTd = nc.dram_tensor("xT", [D, L], F16, kind="ExternalInput")
    wqT = nc.dram_tensor("wqT", [D, E], F16, kind="ExternalInput")
    wkT = nc.dram_tensor("wkT", [D, E], F16, kind="ExternalInput")
    wvT = nc.dram_tensor("wvT", [D, E], F16, kind="ExternalInput")
    woT = nc.dram_tensor("woT", [E, D], F16, kind="ExternalInput")
    trid = nc.dram_tensor("tri", [128, 128], F16, kind="ExternalInput")
    out = nc.dram_tensor("out", [L, D], F16, kind="ExternalOutput")

    with tile.TileContext(nc) as tc:
        with (
            tc.tile_pool(name="const", bufs=1) as constp,
            tc.tile_pool(name="big", bufs=1) as bigp,
            tc.tile_pool(name="qtc", bufs=5) as qtcp,
            tc.tile_pool(name="ptp", bufs=4) as ptp,
            tc.tile_pool(name="smallp", bufs=4) as smallp,
            tc.tile_pool(name="attsbp", bufs=5) as attsbp,
            tc.tile_pool(name="psM", bufs=2, space="PSUM") as psM,
            tc.tile_pool(name="psP", bufs=2, space="PSUM") as psP,
            tc.tile_pool(name="psS", bufs=2, space="PSUM") as psS,
        ):
            ident_h = constp.tile([128, 128], F16)
            make_identity(nc, ident_h[:])
            tri = constp.tile([128, 128], F16)
            nc.sync.dma_start(tri[:], trid[:])

            xT = bigp.tile([128, ND, L], MM)       # x^T  [d-in-tile, d-tile, l]
            WTq = bigp.tile([128, ND, E], MM)      # Wq^T [d-in-tile, d-tile, e]
            WTk = bigp.tile([128, ND, E], MM)
            WTv = bigp.tile([128, ND, E], MM)
            KT = bigp.tile([128, NE, L], MM)       # K^T  [dk (2 heads), e-tile, k]
            attT = bigp.tile([128, NDO, L], MM)
            WoT = bigp.tile([128, NDO, D], MM)
            Vaug = bigp.tile([128, NK, 8, 65], MM)  # V natural per (k-tile, head) + ones

            nc.vector.memset(Vaug[:, :, :, 64:65], 1.0)

            # ---------- input DMAs, dependency order ----------
            # the latency-critical first transfers are issued round-robin
            # across four engine queues: parallel descriptor issue + parallel
            # DMA rings instead of ~610ns serialized per trigger on Sync
            qs = [nc.sync, nc.scalar, nc.gpsimd]

            def dma_w(dst, src, rr=False):
                for dt in range(ND):
                    eng = qs[dt % 3] if rr else nc.sync
                    eng.dma_start(
                        dst[:, dt, :], src[dt * 128:(dt + 1) * 128, :]
                    )

            def dma_x(jc, rr=False):
                for dt in range(ND):
                    eng = qs[dt % 3] if rr else nc.sync
                    eng.dma_start(
                        xT[:, dt, jc * 512:(jc + 1) * 512],
                        xTd[dt * 128:(dt + 1) * 128, jc * 512:(jc + 1) * 512],
                    )

            # critical pieces interleaved in consumer order, one queue per
            # tensor: the first k-proj matmul's two pieces are each
            # first-in-queue instead of queued behind sibling pieces
            for dt in range(ND):
                nc.sync.dma_start(
                    WTk[:, dt, :], wkT[dt * 128:(dt + 1) * 128, :]
                )
                nc.scalar.dma_start(
                    xT[:, dt, 0:512], xTd[dt * 128:(dt + 1) * 128, 0:512]
                )
                nc.gpsimd.dma_start(
                    WTq[:, dt, :], wqT[dt * 128:(dt + 1) * 128, :]
                )
            dma_w(WTv, wvT)
            for jc in range(1, NJ):
                dma_x(jc)
            for et8 in range(NDO):
                nc.sync.dma_start(
                    WoT[:, et8, :], woT[et8 * 128:(et8 + 1) * 128, :]
                )

            # ---------- emission helpers ----------
            def k_proj_group(et, jc):
                def gen():
                    pp = psP.tile([128, 512], F32, tag="pp", name="pp")
                    for dt in range(ND):
                        yield lambda dt=dt, pp=pp: nc.tensor.matmul(
                            pp[:],
                            WTk[:, dt, et * 128:(et + 1) * 128],
                            xT[:, dt, jc * 512:(jc + 1) * 512],
                            start=(dt == 0),
                            stop=(dt == ND - 1),
                        )
                    yield lambda pp=pp: nc.vector.tensor_copy(
                        KT[:, et, jc * 512:(jc + 1) * 512], pp[:]
                    )
                return gen()

            def v_proj_group(lt):
                def gen():
                    pp = psP.tile([128, 512], F32, tag="pp", name="pp")
                    for dt in range(ND):
                        yield lambda dt=dt, pp=pp: nc.tensor.matmul(
                            pp[:],
                            xT[:, dt, lt * 128:(lt + 1) * 128],
                            WTv[:, dt, :],
                            start=(dt == 0),
                            stop=(dt == ND - 1),
                        )
                    yield lambda pp=pp: nc.vector.tensor_copy(
                        Vaug[:, lt, :, 0:64], pp[:]
                    )
                return gen()

            def q_proj_group(j, qtile, et):
                def gen():
                    pp = psP.tile([128, 512], F32, tag="pp", name="pp")
                    for dt in range(ND):
                        yield lambda dt=dt, pp=pp: nc.tensor.matmul(
                            pp[:],
                            WTq[:, dt, et * 128:(et + 1) * 128],
                            xT[:, dt, j * 512:(j + 1) * 512],
                            start=(dt == 0),
                            stop=(dt == ND - 1),
                        )
                    yield lambda pp=pp: nc.vector.tensor_copy(qtile[:, et, :], pp[:])
                return gen()

            def out_proj_group(lt, ec, use_act=False):
                def gen():
                    op = psP.tile([128, 512], F32, tag="pp", name="op")
                    for dt in range(NDO):
                        yield lambda dt=dt, op=op: nc.tensor.matmul(
                            op[:],
                            attT[:, dt, lt * 128:(lt + 1) * 128],
                            WoT[:, dt, ec * 512:(ec + 1) * 512],
                            start=(dt == 0),
                            stop=(dt == NDO - 1),
                        )
                    def tail(op=op):
                        ot = smallp.tile([128, 512], F16, tag="ot", name="ot")
                        # post-exp tail: the scalar engine is idle, use it
                        # for the PSUM drain instead of queueing on DVE
                        if use_act:
                            nc.scalar.copy(ot[:], op[:])
                        else:
                            nc.vector.tensor_copy(ot[:], op[:])
                        nc.sync.dma_start(
                            out[lt * 128:(lt + 1) * 128, ec * 512:(ec + 1) * 512],
                            ot[:],
                        )
                    yield tail
                return gen()

            def chain(gens):
                for g in gens:
                    yield from g

            def drain(it, n):
                k = 0
                for f in it:
                    f()
                    k += 1
                    if k >= n:
                        return

            # ---------- one-unit-deep AV/epilogue pipeline state ----------
            # pend = (j, h, PT) whose AV has not run yet
            state = {"pend": None}
            apair = {}

            def av_steps(jh):
                """Yield closures: 4 AV accumulation groups, then the
                epilogue (reciprocal + normalize + pair transposes)."""
                j, h, PT = jh
                hp = (h % 2) * 64
                hb = h // 2
                if h % 2 == 0:
                    apair[j] = attsbp.tile(
                        [128, 4, 128], MM, tag="apair", name="apair"
                    )
                att_pair = apair[j]
                att_ps = psM.tile([128, 4, 65], F32, tag="m", name="att_ps")

                def group(qt):
                    nq = 4 * j + qt + 1
                    for kt in range(nq):
                        nc.tensor.matmul(
                            att_ps[:, qt, :],
                            PT[:, kt, qt * 128:(qt + 1) * 128],
                            Vaug[:, kt, h, 0:65],
                            start=(kt == 0),
                            stop=(kt == nq - 1),
                        )

                for qt in range(4):
                    yield lambda qt=qt: group(qt)

                def epilogue():
                    rc = smallp.tile([128, 4, 1], F32, tag="rc", name="rc")
                    nc.vector.reciprocal(rc[:], att_ps[:, :, 64:65])
                    nc.vector.tensor_mul(
                        att_pair[:, :, hp:hp + 64],
                        att_ps[:, :, 0:64],
                        rc[:, :, 0:1].to_broadcast((128, 4, 64)),
                    )
                    if h % 2 == 1:
                        for qt in range(4):
                            tpa = psM.tile([128, 128], F16, tag="m", name="tpa")
                            nc.tensor.transpose(
                                tpa[:], att_pair[:, qt, :], ident_h[:]
                            )
                            nc.vector.tensor_copy(
                                attT[:, hb, j * 512 + qt * 128:j * 512 + (qt + 1) * 128],
                                tpa[:],
                            )
                yield epilogue

            # ---------- prologue: just K/Q of chunk 0 for head-pair 0 ----------
            qtiles = {0: qtcp.tile([128, NE, 512], MM, tag="qt", name="qt0")}
            drain(k_proj_group(0, 0), 99)
            drain(q_proj_group(0, qtiles[0], 0), 99)

            def q_alloc(jj):
                def gen():
                    def do():
                        if jj not in qtiles:
                            qtiles[jj] = qtcp.tile(
                                [128, NE, 512], MM, tag="qt", name=f"qt{jj}"
                            )
                    yield do
                return gen()

            def q_proj_lazy(jj, et):
                def gen():
                    pp = psP.tile([128, 512], F32, tag="pp", name="pp")
                    for dt in range(ND):
                        yield lambda dt=dt, pp=pp: nc.tensor.matmul(
                            pp[:],
                            WTq[:, dt, et * 128:(et + 1) * 128],
                            xT[:, dt, jj * 512:(jj + 1) * 512],
                            start=(dt == 0),
                            stop=(dt == ND - 1),
                        )
                    yield lambda pp=pp: nc.vector.tensor_copy(
                        qtiles[jj][:, et, :], pp[:]
                    )
                return gen()

            # ---------- flat interleaved unit schedule ----------
            # Units (j, h) are emitted so chunk j+1's early heads overlap
            # chunk j's late heads: exp work flows into the scalar engine's
            # otherwise-idle early window.
            unit_order = [(j, h) for j in range(NJ) for h in range(8)]
            uidx = {u: i for i, u in enumerate(unit_order)}

            # gated filler segments: (chain, min unit index before draining)
            seg_specs = []
            c0x = [v_proj_group(lt) for lt in range(4)]
            for et in range(1, NE):
                c0x.append(k_proj_group(et, 0))
                c0x.append(q_proj_group(0, qtiles[0], et))
            seg_specs.append((chain(c0x), 0))
            for jj in (1, 2, 3):
                g = [k_proj_group(et, jj) for et in range(NE)]
                g.append(q_alloc(jj))
                g += [q_proj_lazy(jj, et) for et in range(NE)]
                g += [v_proj_group(lt) for lt in range(4 * jj, 4 * jj + 4)]
                seg_specs.append((chain(g), 0))
            # op segments, gated on the pending head that writes the last
            # attT stripe of the source chunk having fully completed
            op03 = chain([out_proj_group(lt, ec) for lt in range(0, 4) for ec in range(2)])
            op47 = chain([out_proj_group(lt, ec) for lt in range(4, 8) for ec in range(2)])
            op811 = chain([out_proj_group(lt, ec) for lt in range(8, 12) for ec in range(2)])
            segments = [
                seg_specs[0],            # c0 extras          (90 items)
                seg_specs[1],            # KQV(1)             (109 items)
                seg_specs[2],            # KQV(2)             (109)
                (op03, uidx[(1, 1)]),    # op rows 0-511      (40)
                seg_specs[3],            # KQV(3)             (109)
                (op47, uidx[(2, 4)]),    # op rows 512-1023   (40)
                (op811, uidx[(3, 4)]),   # op rows 1024-1535  (40)
            ]
            seg_state = {"i": 0, "done": 0}

            def pull_one(unit_i):
                while seg_state["i"] < len(segments):
                    it, gate = segments[seg_state["i"]]
                    if unit_i < gate:
                        return False
                    f = next(it, None)
                    if f is None:
                        seg_state["i"] += 1
                        continue
                    f()
                    seg_state["done"] += 1
                    return True
                return False

            def drain_n(n, unit_i):
                for _ in range(n):
                    if not pull_one(unit_i):
                        return

            def drain_to(target, unit_i):
                while seg_state["done"] < target:
                    if not pull_one(unit_i):
                        return

            # items that must be complete before a unit's scores/AV start
            watermarks = {
                (0, 1): 36, (0, 2): 54, (0, 4): 72, (0, 6): 90,
                (1, 0): 199, (2, 0): 308, (3, 0): 457,
            }

            # filler pacing: piecewise-linear in global slot count, anchored
            # to the dependency watermarks so no big force-drain bursts occur
            anchors = [(0, 0), (16, 199), (48, 308), (96, 457), (130, 505), (160, 537)]

            def target_at(s):
                for (s0, t0), (s1, t1) in zip(anchors, anchors[1:]):
                    if s <= s1:
                        return t0 + (t1 - t0) * (s - s0) / (s1 - s0)
                return anchors[-1][1]

            gs = 0  # global slot counter
            carry_pend = None  # previous head's leftover AV steps, drained
                               # after the next head's first score pair so
                               # score production is back-to-back across
                               # head boundaries (keeps ACT fed)

            for ui, (j, h) in enumerate(unit_order):
                nkt = 4 * (j + 1)
                hp = (h % 2) * 64
                hb = h // 2
                if (j, h) in watermarks:
                    drain_to(watermarks[(j, h)], ui)
                QTc_ = qtiles[j]
                PT = ptp.tile([128, NK, 512], MM, tag="pt", name="pt")
                pend_steps = (
                    av_steps(state["pend"]) if state["pend"] is not None else None
                )
                n_pend = 5 if pend_steps is not None else 0
                pend_acc = 0.0
                pend_done = 0
                slots = nkt // 2
                is_last = ui == len(unit_order) - 1
                for sl_ in range(slots):
                    # last unit: full-width first so the trailing exps are
                    # the small diagonal ones and its AV unblocks sooner
                    sl = (sl_ + 2) % slots if is_last else sl_
                    s_ps = psS.tile([128, 2, 512], F32, tag="s", name="s_ps")
                    if sl >= 2:
                        # full-width k-tile pair (diag pairs go first so the
                        # big exp tiles trail across the unit boundary)
                        for half in range(2):
                            kt = 2 * (sl - 2) + half
                            nc.tensor.matmul(
                                s_ps[:, half, :],
                                KT[hp:hp + 64, hb, kt * 128:(kt + 1) * 128],
                                QTc_[hp:hp + 64, hb, :],
                            )
                        nc.scalar.activation(
                            PT[:, 2 * (sl - 2):2 * (sl - 2) + 2, :],
                            s_ps[:],
                            AF.Exp,
                        )
                    else:
                        # diagonal pair at 128-col causal granularity
                        for half in range(2):
                            m = 2 * sl + half
                            kt = nkt - 4 + m
                            nc.tensor.matmul(
                                s_ps[:, half, 128 * m:512],
                                KT[hp:hp + 64, hb, kt * 128:(kt + 1) * 128],
                                QTc_[hp:hp + 64, hb, 128 * m:512],
                            )
                        for half in range(2):
                            m = 2 * sl + half
                            kt = nkt - 4 + m
                            nc.scalar.activation(
                                PT[:, kt, 128 * m:512],
                                s_ps[:, half, 128 * m:512],
                                AF.Exp,
                            )
                            nc.vector.tensor_mul(
                                PT[:, kt, 128 * m:128 * (m + 1)],
                                PT[:, kt, 128 * m:128 * (m + 1)],
                                tri[:],
                            )
                    # leftover AV/epilogue carried across the boundary
                    if sl_ == 0 and carry_pend is not None:
                        drain(carry_pend, 99)
                        carry_pend = None
                    gs += 1
                    if sl_ < slots - 1:
                        # interleave the pending unit's AV/epilogue
                        pend_acc += n_pend / slots
                        take = int(pend_acc) - pend_done
                        if take > 0 and pend_steps is not None:
                            drain(pend_steps, take)
                            pend_done += take
                        # smear projection / output-projection filler
                        drain_to(int(target_at(gs)), ui)
                carry_pend = pend_steps
                state["pend"] = (j, h, PT)

            if carry_pend is not None:
                drain(carry_pend, 99)
            drain_to(10 ** 9, len(unit_order))
            drain(av_steps(state["pend"]), 99)
            for lt in range(4 * (NJ - 1), 4 * (NJ - 1) + 4):
                for ec in range(2):
                    drain(out_proj_group(lt, ec, use_act=True), 99)

    nc.compile()
    return nc


def _get_program():
    if "nc" not in _CACHE:
        _CACHE["nc"] = build_program()
    return _CACHE["nc"]


def make_in_maps(x, Wq, Wk, Wv, Wo):
    x = np.asarray(x, dtype=np.float32)
    Wq = np.asarray(Wq, dtype=np.float32)
    Wk = np.asarray(Wk, dtype=np.float32)
    Wv = np.asarray(Wv, dtype=np.float32)
    Wo = np.asarray(Wo, dtype=np.float32)
    tri = (np.arange(128)[None, :] >= np.arange(128)[:, None]).astype(np.float16)
    in_maps = []
    for c in range(8):
        b, hg = c // 2, c % 2
        sl = slice(hg * E, (hg + 1) * E)
        in_maps.append(
            {
                "xT": x[b].T.astype(np.float16),
                "wqT": Wq[sl].T.astype(np.float16),
                "wkT": (Wk[sl] * 0.125).T.astype(np.float16),
                "wvT": Wv[sl].T.astype(np.float16),
                "woT": Wo[:, sl].T.astype(np.float16),
                "tri": tri,
            }
        )
    return in_maps


def kernel(x, Wq, Wk, Wv, Wo, **run_kwargs):
    from concourse import bass_utils

    nc = _get_program()
    in_maps = make_in_maps(x, Wq, Wk, Wv, Wo)
    res = bass_utils.run_bass_kernel_spmd(
        nc, in_maps, core_ids=list(range(8)), **run_kwargs
    )
    o = np.empty((B, L, D), np.float32)
    for b in range(B):
        o[b] = res.results[2 * b]["out"].astype(np.float32) + res.results[
            2 * b + 1
        ]["out"].astype(np.float32)
    _CACHE["last_result"] = res
    return o
